# revision 2
# baseline (speedup 1.0000x reference)
"""Trainium2 kernel for nn_Direction: out = input @ Q.T, Q from QR(weight + 1e-8).

Strategy (v2):
  - Host: QR of the small 512x512 weight (fp32), pre-transpose each batch
    shard so the contraction dim (motion=512) lands on SBUF partitions, cast
    A.T and Q.T to fp16 (single pass: the harness gate is 2e-2 rel err and
    fp16 rounding contributes only ~5e-4).
  - Device (8 cores, data-parallel over batch): Q.T tiles are the STATIONARY
    operand (16 tiles of 128x128, SBUF-resident constants); A.T streams as
    the moving operand in 512-column chunks. For each (n-tile, k-tile) the
    stationary is loaded once and 8 consecutive matmuls reuse it, so the
    128-col LDWEIGHTS is amortized (the v1 kernel reloaded a fresh A-tile
    stationary before every matmul, paying ~60ns/MM extra).
  - Output is produced transposed (out.T[n, b] in PSUM with n on partitions),
    evicted fp32->fp16 on alternating scalar/vector engines, DMA'd to a
    [512, b_loc] fp16 DRAM tensor; the host un-transposes and upcasts.
  - fp16 output halves the HBM write traffic, keeping total DMA (~94us/core)
    under the single-pass PE streaming time (~112us/core).
"""

import numpy as np

import concourse.bacc as bacc
import concourse.mybir as mybir
import concourse.tile as tile
from concourse.bass_utils import run_bass_kernel_spmd

B_FULL = 131072
D = 512
N_CORES = 8
B_LOC = B_FULL // N_CORES  # 16384
P = 128
KT = D // P  # 4 k-tiles (contraction)
NT = D // P  # 4 n-tiles (output dim)

MODE = "fp16q"

_CACHE = {}


def _build(mode, b_loc, reps=1, dynamic=False, gb=4096, ch=512, korder="kc",
           ain_bufs=2, aout_bufs=3, ps_bufs=8, evict="alt"):
    """mode: 'fp16q' (fp16 single pass, Q stationary) or 'bf16q'.
    korder: 'kc' = k outer / chunk inner (8 consecutive matmuls share the
    stationary); 'ck' = chunk outer / k inner (stationary changes every MM,
    v1-style, kept for A/B measurement)."""
    dt_in = {"fp16q": mybir.dt.float16, "bf16q": mybir.dt.bfloat16}[mode]
    ng = b_loc // gb
    nch = gb // ch

    nc = bacc.Bacc("TRN2", target_bir_lowering=False, debug=False)
    a_dram = nc.dram_tensor("a0", [D, b_loc], dt_in, kind="ExternalInput").ap()
    q_dram = nc.dram_tensor("q0", [D, D], dt_in, kind="ExternalInput").ap()
    out_dram = nc.dram_tensor(
        "out", [D, b_loc], dt_in, kind="ExternalOutput"
    ).ap()

    with tile.TileContext(nc) as tc:
        with (
            tc.tile_pool(name="consts", bufs=1) as consts,
            tc.tile_pool(name="ain", bufs=ain_bufs) as ain,
            tc.tile_pool(name="aout", bufs=aout_bufs) as aout,
            tc.tile_pool(name="ps", bufs=ps_bufs, space="PSUM") as ps_pool,
        ):
            qt = consts.tile([P, KT, D], dt_in, name="qt")
            nc.sync.dma_start(
                out=qt[:, :, :],
                in_=q_dram.rearrange("(k p) n -> p k n", p=P),
            )

            def body():
                for g in range(ng):
                    at = ain.tile([P, KT, gb], dt_in, name="at", tag="at")
                    src = a_dram.rearrange("(k p) b -> p k b", p=P)[
                        :, :, g * gb : (g + 1) * gb
                    ]
                    nc.sync.dma_start(out=at[:, :, :], in_=src)
                    for n in range(NT):
                        ot = aout.tile([P, gb], dt_in, name="ot", tag="ot")
                        pss = [
                            ps_pool.tile(
                                [P, ch], mybir.dt.float32, name="ps", tag="ps"
                            )
                            for _ in range(nch)
                        ]
                        if korder == "kc":
                            for k in range(KT):
                                lhsT = qt[:, k, n * P : (n + 1) * P]
                                for c in range(nch):
                                    nc.tensor.matmul(
                                        pss[c][:, :],
                                        lhsT,
                                        at[:, k, c * ch : (c + 1) * ch],
                                        start=(k == 0),
                                        stop=(k == KT - 1),
                                    )
                        else:
                            for c in range(nch):
                                for k in range(KT):
                                    nc.tensor.matmul(
                                        pss[c][:, :],
                                        qt[:, k, n * P : (n + 1) * P],
                                        at[:, k, c * ch : (c + 1) * ch],
                                        start=(k == 0),
                                        stop=(k == KT - 1),
                                    )
                        for c in range(nch):
                            dst = ot[:, c * ch : (c + 1) * ch]
                            if evict == "alt":
                                if c % 2 == 0:
                                    nc.scalar.activation(
                                        dst,
                                        pss[c][:, :],
                                        mybir.ActivationFunctionType.Copy,
                                    )
                                else:
                                    nc.vector.tensor_copy(dst, pss[c][:, :])
                            elif evict == "vector":
                                nc.vector.tensor_copy(dst, pss[c][:, :])
                            else:
                                nc.any.tensor_copy(dst, pss[c][:, :])
                        nc.sync.dma_start(
                            out=out_dram[n * P : (n + 1) * P, g * gb : (g + 1) * gb],
                            in_=ot[:, :],
                        )

            if dynamic and reps > 1:
                with tc.For_i(0, reps, 1):
                    body()
            else:
                for _ in range(reps):
                    body()

    nc.compile()
    return nc


def _get_nc(mode, b_loc):
    return _get_nc_reps(mode, b_loc, 1)


def _get_nc_reps(mode, b_loc, reps, dynamic=False, **kw):
    key = (mode, b_loc, reps, dynamic, tuple(sorted(kw.items())))
    if key not in _CACHE:
        _CACHE[key] = _build(mode, b_loc, reps, dynamic, **kw)
    return _CACHE[key]


def _prep_inputs(mode, input_np, qt_np, n_cores, b_loc):
    """Build per-core input maps. input_np: (n_cores*b_loc, D) fp32 row-major.
    qt_np: (D, D) fp32, qt_np[m, n] = Q[n, m]."""
    import ml_dtypes

    cast_dt = {"fp16q": np.float16, "bf16q": ml_dtypes.bfloat16}[mode]
    q0 = qt_np.astype(cast_dt)
    maps = []
    for i in range(n_cores):
        at = np.ascontiguousarray(input_np[i * b_loc : (i + 1) * b_loc].T).astype(
            cast_dt
        )
        maps.append({"a0": at, "q0": q0})
    return maps


def _compute_qt(weight_np):
    """Q from QR(weight + 1e-8), transposed. Prefer jax-on-CPU so Q matches the
    fp32 jax reference bit-for-bit when possible; fall back to LAPACK (both are
    Householder QR and agree to ~1e-6, so either is well within tolerance)."""
    w = weight_np.astype(np.float32)
    try:
        import jax
        import jax.numpy as jnp

        cpu = jax.devices("cpu")[0]
        with jax.default_device(cpu):
            q, _ = jnp.linalg.qr(jax.device_put(w, cpu) + 1e-8)
        q = np.asarray(q)
    except Exception:
        q, _ = np.linalg.qr(w + np.float32(1e-8))
    return np.ascontiguousarray(q.T.astype(np.float32))


def run(input_np, weight_np, mode=None, n_cores=N_CORES, b_loc=None, **run_kwargs):
    mode = mode or MODE
    b_loc = b_loc or (input_np.shape[0] // n_cores)
    assert input_np.shape[0] == n_cores * b_loc, (
        f"batch {input_np.shape[0]} not divisible into {n_cores} cores"
    )
    assert input_np.shape[1] == D

    qt = _compute_qt(weight_np)

    nc = _get_nc(mode, b_loc)
    in_maps = _prep_inputs(mode, np.asarray(input_np), qt, n_cores, b_loc)
    res = run_bass_kernel_spmd(nc, in_maps, list(range(n_cores)), **run_kwargs)
    out = np.empty((n_cores * b_loc, D), dtype=np.float32)
    for i in range(n_cores):
        out[i * b_loc : (i + 1) * b_loc] = (
            np.asarray(res.results[i]["out"]).T.astype(np.float32)
        )
    return out, res


def kernel(input, weight):
    out, _ = run(
        np.asarray(input, dtype=np.float32), np.asarray(weight, dtype=np.float32)
    )
    return np.ascontiguousarray(out, dtype=np.float32)


# revision 19
# speedup vs baseline: 1.0816x; 1.0816x over previous
"""Trainium2 kernel for nn_Direction: out = input @ Q.T, Q from QR(weight + 1e-8).

Strategy (v2, measured ~172us/core vs 418us baseline):
  - Host: QR of the small 512x512 weight (fp32), pre-transpose each batch
    shard so the contraction dim (motion=512) lands on SBUF partitions, cast
    A.T and Q.T to fp16. A SINGLE fp16 pass replaces the baseline's 3-pass
    fp16-split scheme: the harness gate is 2e-2 rel err and fp16 rounding
    contributes only ~4e-4 (measured 3.6e-4), so the extra passes bought
    nothing but 3x PE time.
  - Device (8 cores, data-parallel over batch): Q.T tiles are the stationary
    operand (16 SBUF-resident 128x128 tiles); A.T streams as the moving
    operand in 512-column chunks, accumulated over the 4 k-tiles into one
    PSUM bank per chunk ('ck' order: each bank's 4-matmul accumulation group
    runs contiguously — HW A/B showed bank-interleaved ordering costs ~96us).
  - Output is produced transposed (out.T[n, b] in PSUM with n on partitions),
    evicted fp32->fp16 on alternating scalar/vector engines, DMA'd to a
    [512, b_loc] fp16 DRAM tensor; the host un-transposes and upcasts.
    fp16 output halves HBM write traffic (~94us/core total DMA, overlapped).

HW findings (differential timing, no NTFF under this axon client):
  - Per-matmul cost is ~(400+N)/2.4GHz — the isolated-MM latency; consecutive
    matmuls do not pipeline fill/drain in this structure (N-sweep: ch=512 ->
    379ns/MM, ch=256 -> 294ns/MM, both matching the isolated model).
  - Deduplicating the per-matmul InstLdweights that tile_legalize emits
    (dedup_ldw=1, 512 -> ~100 loads) recovered only ~8-18us: weight reloads
    are NOT the dominant per-MM cost.
  - The baseline's A-stationary structure at a single pass (mode 'fp16a')
    measures 261us — its fp16x2 efficiency came from 12-MM accumulation
    groups, which a single pass cannot have (K=512 = 4 k-tiles only).
"""

import numpy as np

import concourse.bacc as bacc
import concourse.mybir as mybir
import concourse.tile as tile
from concourse.bass_utils import run_bass_kernel_spmd

B_FULL = 131072
D = 512
N_CORES = 8
B_LOC = B_FULL // N_CORES  # 16384
P = 128
KT = D // P  # 4 k-tiles (contraction)
NT = D // P  # 4 n-tiles (output dim)

MODE = "fp16q"

_CACHE = {}


def _dedup_ldweights(obb):
    """Drop InstLdweights that reload the PE array with weights identical to
    the immediately-preceding load (with only plain non-transpose InstMatmult
    in between on the PE stream). tile_legalize mechanically emits one
    Ldweights per Matmult even when 8 consecutive matmuls share the same
    stationary operand; the PE weight registers persist, so the reloads are
    pure overhead (~60ns each serialized ahead of every 213ns matmul).

    Runs on tile_legalize's output, BEFORE semaphore assignment, so all sync
    bookkeeping is computed on the filtered stream. Dependency edges of a
    dropped load are merged into the retained one.
    """
    n_removed = 0
    for bb, insts in obb.items():
        out = []
        last_ld = None
        last_key = None
        for inst in insts:
            tn = type(inst).__name__
            if tn == "InstLdweights":
                if any(inst.regs_accessed()):
                    # register-dependent AP: identity not static, keep
                    out.append(inst)
                    last_ld, last_key = None, None
                    continue
                key = (
                    str(inst.ins[0]),
                    str(inst.perf_mode),
                    str(inst.tile_position),
                    bool(inst.is_transpose),
                )
                if last_key == key and last_ld is not None:
                    last_ld.merge_dependencies_from(inst)
                    n_removed += 1
                    continue
                last_ld, last_key = inst, key
                out.append(inst)
            elif tn == "InstMatmult":
                if inst.is_transpose or inst.perf_mode is not None:
                    last_ld, last_key = None, None
                out.append(inst)
            else:
                if last_ld is not None and inst.engine == last_ld.engine:
                    last_ld, last_key = None, None
                out.append(inst)
        obb[bb] = out
    return n_removed


def _build(mode, b_loc, reps=1, dynamic=False, gb=4096, ch=512, korder="ck",
           ain_bufs=2, aout_bufs=3, ps_bufs=8, evict="alt", variant="full",
           dedup_ldw=0):
    """mode: 'fp16q' (fp16 single pass, Q stationary) or 'bf16q'.
    korder: 'kc' = k outer / chunk inner (8 consecutive matmuls share the
    stationary); 'ck' = chunk outer / k inner (stationary changes every MM,
    v1-style, kept for A/B measurement).
    variant (timing experiments only, output garbage unless 'full'):
      'mmonly' = no A DMA / evict / out-DMA (pure PE stream);
      'noevict' = A DMA + matmuls only."""
    dt_in = {"fp16q": mybir.dt.float16, "bf16q": mybir.dt.bfloat16}[mode]
    ng = b_loc // gb
    nch = gb // ch

    nc = bacc.Bacc("TRN2", target_bir_lowering=False, debug=False)
    a_dram = nc.dram_tensor("a0", [D, b_loc], dt_in, kind="ExternalInput").ap()
    q_dram = nc.dram_tensor("q0", [D, D], dt_in, kind="ExternalInput").ap()
    out_dram = nc.dram_tensor(
        "out", [D, b_loc], dt_in, kind="ExternalOutput"
    ).ap()

    orig_legalize = tile.tile_legalize
    if dedup_ldw:
        def _patched_legalize(obb, nc_):
            obb = orig_legalize(obb, nc_)
            _dedup_ldweights(obb)
            return obb

        tile.tile_legalize = _patched_legalize
    try:
        _build_body(nc, a_dram, q_dram, out_dram, mode, b_loc, reps, dynamic,
                    gb, ch, korder, ain_bufs, aout_bufs, ps_bufs, evict,
                    variant)
    finally:
        tile.tile_legalize = orig_legalize

    nc.compile()
    return nc


def _build_body(nc, a_dram, q_dram, out_dram, mode, b_loc, reps, dynamic, gb,
                ch, korder, ain_bufs, aout_bufs, ps_bufs, evict, variant):
    dt_in = {"fp16q": mybir.dt.float16, "bf16q": mybir.dt.bfloat16}[mode]
    ng = b_loc // gb
    nch = gb // ch

    with tile.TileContext(nc) as tc:
        with (
            tc.tile_pool(name="consts", bufs=1) as consts,
            tc.tile_pool(name="ain", bufs=ain_bufs) as ain,
            tc.tile_pool(name="aout", bufs=aout_bufs) as aout,
            tc.tile_pool(name="ps", bufs=ps_bufs, space="PSUM") as ps_pool,
        ):
            qt = consts.tile([P, KT, D], dt_in, name="qt")
            nc.sync.dma_start(
                out=qt[:, :, :],
                in_=q_dram.rearrange("(k p) n -> p k n", p=P),
            )
            rc = None
            if variant == "mmonly":
                # dedicated moving-operand tile (NOT qt: the LDW read and the
                # rhs stream must not hit the same SBUF region)
                rc = consts.tile([P, KT, ch], dt_in, name="rc")
                nc.sync.dma_start(
                    out=rc[:, :, :],
                    in_=a_dram.rearrange("(k p) b -> p k b", p=P)[:, :, 0:ch],
                )

            def body():
                for g in range(ng):
                    if variant == "mmonly":
                        at = None
                    else:
                        at = ain.tile([P, KT, gb], dt_in, name="at", tag="at")
                        src = a_dram.rearrange("(k p) b -> p k b", p=P)[
                            :, :, g * gb : (g + 1) * gb
                        ]
                        nc.sync.dma_start(out=at[:, :, :], in_=src)
                    for n in range(NT):
                        pss = [
                            ps_pool.tile(
                                [P, ch], mybir.dt.float32, name="ps", tag="ps"
                            )
                            for _ in range(nch)
                        ]

                        def rhs(k, c):
                            if at is None:
                                return rc[:, k, :]
                            return at[:, k, c * ch : (c + 1) * ch]

                        if korder == "kc":
                            for k in range(KT):
                                lhsT = qt[:, k, n * P : (n + 1) * P]
                                for c in range(nch):
                                    nc.tensor.matmul(
                                        pss[c][:, :],
                                        lhsT,
                                        rhs(k, c),
                                        start=(k == 0),
                                        stop=(k == KT - 1),
                                    )
                        else:
                            for c in range(nch):
                                for k in range(KT):
                                    nc.tensor.matmul(
                                        pss[c][:, :],
                                        qt[:, k, n * P : (n + 1) * P],
                                        rhs(k, c),
                                        start=(k == 0),
                                        stop=(k == KT - 1),
                                    )
                        if variant in ("mmonly", "noevict"):
                            continue
                        ot = aout.tile([P, gb], dt_in, name="ot", tag="ot")
                        for c in range(nch):
                            dst = ot[:, c * ch : (c + 1) * ch]
                            if evict == "alt":
                                if c % 2 == 0:
                                    nc.scalar.activation(
                                        dst,
                                        pss[c][:, :],
                                        mybir.ActivationFunctionType.Copy,
                                    )
                                else:
                                    nc.vector.tensor_copy(dst, pss[c][:, :])
                            elif evict == "vector":
                                nc.vector.tensor_copy(dst, pss[c][:, :])
                            else:
                                nc.any.tensor_copy(dst, pss[c][:, :])
                        nc.sync.dma_start(
                            out=out_dram[n * P : (n + 1) * P, g * gb : (g + 1) * gb],
                            in_=ot[:, :],
                        )

            if dynamic and reps > 1:
                with tc.For_i(0, reps, 1):
                    body()
            else:
                for _ in range(reps):
                    body()


def _build_a(mode, b_loc, reps=1, dynamic=False, bt=512, ain_bufs=3,
             aout_bufs=3, ps_bufs=8, evict="any"):
    """v1 structure: A.T tiles stationary (reloaded every matmul), Q.T moving,
    out[b, n] fp32 directly.  mode 'fp16a' = single fp16 pass."""
    dt_in = mybir.dt.float16
    n_iter = b_loc // bt
    sb_n = bt // P

    nc = bacc.Bacc("TRN2", target_bir_lowering=False, debug=False)
    a_dram = nc.dram_tensor("a0", [D, b_loc], dt_in, kind="ExternalInput").ap()
    q_dram = nc.dram_tensor("q0", [D, D], dt_in, kind="ExternalInput").ap()
    out_dram = nc.dram_tensor(
        "out", [b_loc, D], mybir.dt.float32, kind="ExternalOutput"
    ).ap()

    with tile.TileContext(nc) as tc:
        with (
            tc.tile_pool(name="consts", bufs=1) as consts,
            tc.tile_pool(name="ain", bufs=ain_bufs) as ain,
            tc.tile_pool(name="aout", bufs=aout_bufs) as aout,
            tc.tile_pool(name="ps", bufs=ps_bufs, space="PSUM") as ps_pool,
        ):
            qt = consts.tile([P, KT, D], dt_in, name="qt")
            nc.sync.dma_start(
                out=qt[:, :, :],
                in_=q_dram.rearrange("(k p) n -> p k n", p=P),
            )

            def body():
                for it in range(n_iter):
                    at = ain.tile([P, KT, bt], dt_in, name="at", tag="at")
                    src = a_dram.rearrange("(k p) b -> p k b", p=P)[
                        :, :, it * bt : (it + 1) * bt
                    ]
                    nc.sync.dma_start(out=at[:, :, :], in_=src)
                    for sb in range(sb_n):
                        ps = ps_pool.tile(
                            [P, D], mybir.dt.float32, name="ps", tag="ps"
                        )
                        for k in range(KT):
                            nc.tensor.matmul(
                                ps[:, :],
                                at[:, k, sb * P : (sb + 1) * P],
                                qt[:, k, :],
                                start=(k == 0),
                                stop=(k == KT - 1),
                            )
                        ot = aout.tile([P, D], mybir.dt.float32, name="ot",
                                       tag="ot")
                        if evict == "any":
                            nc.any.tensor_copy(ot[:, :], ps[:, :])
                        elif evict == "vector":
                            nc.vector.tensor_copy(ot[:, :], ps[:, :])
                        else:
                            if sb % 2 == 0:
                                nc.vector.tensor_copy(ot[:, :], ps[:, :])
                            else:
                                nc.scalar.activation(
                                    ot[:, :],
                                    ps[:, :],
                                    mybir.ActivationFunctionType.Copy,
                                )
                        b0 = it * bt + sb * P
                        nc.sync.dma_start(
                            out=out_dram[b0 : b0 + P, :], in_=ot[:, :]
                        )

            if dynamic and reps > 1:
                with tc.For_i(0, reps, 1):
                    body()
            else:
                for _ in range(reps):
                    body()

    nc.compile()
    return nc


def _get_nc(mode, b_loc):
    return _get_nc_reps(mode, b_loc, 1)


def _get_nc_reps(mode, b_loc, reps, dynamic=False, **kw):
    key = (mode, b_loc, reps, dynamic, tuple(sorted(kw.items())))
    if key not in _CACHE:
        if mode == "fp16a":
            _CACHE[key] = _build_a(mode, b_loc, reps, dynamic, **kw)
        else:
            _CACHE[key] = _build(mode, b_loc, reps, dynamic, **kw)
    return _CACHE[key]


def _prep_inputs(mode, input_np, qt_np, n_cores, b_loc):
    """Build per-core input maps. input_np: (n_cores*b_loc, D) fp32 row-major.
    qt_np: (D, D) fp32, qt_np[m, n] = Q[n, m]."""
    if mode == "bf16q":
        import ml_dtypes

        cast_dt = ml_dtypes.bfloat16
    else:
        cast_dt = np.float16
    q0 = qt_np.astype(cast_dt)
    maps = []
    for i in range(n_cores):
        at = np.ascontiguousarray(input_np[i * b_loc : (i + 1) * b_loc].T).astype(
            cast_dt
        )
        maps.append({"a0": at, "q0": q0})
    return maps


def _compute_qt(weight_np):
    """Q from QR(weight + 1e-8), transposed. Prefer jax-on-CPU so Q matches the
    fp32 jax reference bit-for-bit when possible; fall back to LAPACK (both are
    Householder QR and agree to ~1e-6, so either is well within tolerance)."""
    w = weight_np.astype(np.float32)
    try:
        import jax
        import jax.numpy as jnp

        cpu = jax.devices("cpu")[0]
        with jax.default_device(cpu):
            q, _ = jnp.linalg.qr(jax.device_put(w, cpu) + 1e-8)
        q = np.asarray(q)
    except Exception:
        q, _ = np.linalg.qr(w + np.float32(1e-8))
    return np.ascontiguousarray(q.T.astype(np.float32))


def run(input_np, weight_np, mode=None, n_cores=N_CORES, b_loc=None, **run_kwargs):
    mode = mode or MODE
    b_loc = b_loc or (input_np.shape[0] // n_cores)
    assert input_np.shape[0] == n_cores * b_loc, (
        f"batch {input_np.shape[0]} not divisible into {n_cores} cores"
    )
    assert input_np.shape[1] == D

    qt = _compute_qt(weight_np)

    nc = _get_nc(mode, b_loc)
    in_maps = _prep_inputs(mode, np.asarray(input_np), qt, n_cores, b_loc)
    res = run_bass_kernel_spmd(nc, in_maps, list(range(n_cores)), **run_kwargs)
    out = np.empty((n_cores * b_loc, D), dtype=np.float32)
    for i in range(n_cores):
        o = np.asarray(res.results[i]["out"])
        if mode == "fp16a":
            out[i * b_loc : (i + 1) * b_loc] = o
        else:
            out[i * b_loc : (i + 1) * b_loc] = o.T.astype(np.float32)
    return out, res


def kernel(input, weight):
    out, _ = run(
        np.asarray(input, dtype=np.float32), np.asarray(weight, dtype=np.float32)
    )
    return np.ascontiguousarray(out, dtype=np.float32)
